# revision 2
# baseline (speedup 1.0000x reference)
# Trainium2 Bass kernel for a binarized 2-block MLP (BNN):
#   h1 = sign(BN1(x @ sign(w1).T + b1)); h2 = sign(BN2(h1 @ sign(w2).T + b2))
#   out = log_softmax(h2 @ sign(w5).T + b5)
#
# v4 (on top of v3's feature-major back half):
#   * hi part shipped pre-scaled by 2048 (exact in fp16), so the hi and lo
#     fc1 passes share ONE +-1 weight matrix: 8 LDWEIGHTS + 16 MMs per
#     chunk instead of 16+16. The 2^-11 folds into the BN1 scale
#     (bit-exact: pure exponent shifts).
#   * ln/stt/store lag one extra iteration so ACT never waits on the
#     same-iteration sum matmul; a DVE copy of ps5 -> SBUF keeps the PSUM
#     pools at 8 banks.
#   * ~10 dummy matmuls (gated only on the constants DMA) warm the PE HAM
#     clock to 2.4GHz before real work arrives.
#   * retained: packed consts first, chunk-granular x DMA with chunk 0 in
#     4 k-slabs, last two chunks' compute split in halves, per-chunk
#     feature-major stores.
import os
import sys

import numpy as np

for _p in ("/opt/trn_rl_repo", "/root/.axon_site/_ro/trn_rl_repo"):
    if os.path.isdir(_p) and _p not in sys.path:
        sys.path.insert(0, _p)

import concourse.bass as bass
import concourse.mybir as mybir
import concourse.tile as tile
from concourse import bacc

N_CORES = 8
B, D, H1, H2, O = 65536, 1024, 50, 20, 10
BPC = B // N_CORES  # batch rows per core
CH = 512            # batch chunk (one PSUM bank of fp32)
KS = D // 128       # contraction slices
EPS = 1e-4
LO = 2048.0         # hi-part pre-scale (2**11)

F16 = mybir.dt.float16
BF16 = mybir.dt.bfloat16
F32 = mybir.dt.float32
AF = mybir.ActivationFunctionType
AX = mybir.AxisListType
OP = mybir.AluOpType

# packed fp16 consts layout (columns)
C16_W1 = 0                  # [128, KS*H1]  sign(w1) swizzled
C16_W2 = KS * H1            # rows 0:50, [50, H2]
C16_W5 = C16_W2 + H2        # rows 0:20, [20, O]
C16_N = C16_W5 + O
# packed fp32 consts layout
C32_CS1 = 0                 # rows 0:50, [50, 2]  (scale/2048, shift)
C32_CS2 = 2                 # rows 0:20, [20, 2]
C32_B5 = 4                  # rows 0:10, [10, 1]
C32_N = 5
# packed bf16 consts
CB_ONES = 0                 # rows 0:10, [10, 10] all-ones
CB_N = O


def build_bass(bpc: int = BPC) -> bass.Bass:
    nch = bpc // CH
    nc = bacc.Bacc("TRN2", target_bir_lowering=False)

    # Restrict the ACT-table chooser to the combined set so Sign/Exp/Ln all
    # come from one table load.
    def _act_table_loads_combined_set_only(self=nc):
        import bass_rust as _br

        from concourse.hw_specs import get_activation_tables

        has_act = any(
            isinstance(i, mybir.InstActivation)
            for blk in self.main_func.blocks
            for i in blk.instructions
        )
        if not has_act:
            return
        tabs = get_activation_tables(self.m.arch)
        tables = [
            (name, fns if name == "natural_log_exp_and_others" else set())
            for name, fns in tabs.items()
        ]
        _br.insert_act_table_loads(self, tables)

    nc.insert_act_table_loads = _act_table_loads_combined_set_only

    # x arrives packed: xp[p, c, h, k, n] = part h (0=hi*2048, 1=lo) of
    # x.T[k*128+p, c*CH+n]; per partition a chunk slice is contiguous 16KB.
    xp = nc.declare_dram_parameter("xp", [128, nch, 2, KS, CH], F16, isOutput=False)
    c16 = nc.declare_dram_parameter("c16", [128, C16_N], F16, isOutput=False)
    c32 = nc.declare_dram_parameter("c32", [128, C32_N], F32, isOutput=False)
    cb = nc.declare_dram_parameter("cb", [128, CB_N], BF16, isOutput=False)
    # Output, feature-major: y[o, r] = out[r, o]
    y = nc.declare_dram_parameter("y", [O, bpc], F32, isOutput=True)

    with tile.TileContext(nc) as tc:
        from contextlib import ExitStack

        with ExitStack() as ctx:
            singles = ctx.enter_context(tc.tile_pool(name="singles", bufs=1))
            xpool = ctx.enter_context(tc.tile_pool(name="xpool", bufs=8))
            mids = ctx.enter_context(tc.tile_pool(name="mids", bufs=3))
            p1pool = ctx.enter_context(tc.tile_pool(name="p1", bufs=2, space="PSUM"))
            p2pool = ctx.enter_context(tc.tile_pool(name="p2", bufs=2, space="PSUM"))
            p5pool = ctx.enter_context(tc.tile_pool(name="p5", bufs=2, space="PSUM"))
            pSpool = ctx.enter_context(tc.tile_pool(name="pS", bufs=2, space="PSUM"))

            # constants first: three contiguous partition-major transfers
            c16_sb = singles.tile([128, C16_N], F16)
            nc.sync.dma_start(out=c16_sb, in_=c16[:, :])
            c32_sb = singles.tile([128, C32_N], F32)
            nc.sync.dma_start(out=c32_sb, in_=c32[:, :])
            cb_sb = singles.tile([128, CB_N], BF16)
            nc.sync.dma_start(out=cb_sb, in_=cb[:, :])

            # x stream, chunk granularity; chunk 0 lands as 4 k-slabs so the
            # first matmuls can begin earlier.
            xts = []
            for c in range(nch):
                xt = xpool.tile([128, 2, KS, CH], F16, tag="x", name="x_t")
                nslab = {0: 4}.get(c, 1)
                ks_per = KS // nslab
                for s in range(nslab):
                    sl = slice(s * ks_per, (s + 1) * ks_per)
                    nc.sync.dma_start(out=xt[:, :, sl, :],
                                      in_=xp[:, c, :, sl, :])
                xts.append(xt)

            cs1_s = c32_sb[0:H1, C32_CS1:C32_CS1 + 1]
            cs1_t = c32_sb[0:H1, C32_CS1 + 1:C32_CS1 + 2]
            cs2_s = c32_sb[32:32 + H2, C32_CS2:C32_CS2 + 1]
            cs2_t = c32_sb[32:32 + H2, C32_CS2 + 1:C32_CS2 + 2]
            b5_c = c32_sb[0:O, C32_B5:C32_B5 + 1]
            w2_sb = c16_sb[0:H1, C16_W2:C16_W2 + H2]
            w5_sb = c16_sb[32:32 + H2, C16_W5:C16_W5 + O]
            ones_sb = cb_sb[0:O, CB_ONES:CB_ONES + O]

            olog = singles.tile([O, bpc], F32)

            # PE warmup: dummy matmuls gated only on the consts DMA keep the
            # HAM activity window busy while the first x slabs land, so real
            # work starts at 2.4GHz. Output goes to the pS ring (never read).
            warm = pSpool.tile([O, CH], F32, tag="psS", name="warm")
            for _ in range(10):
                nc.tensor.matmul(warm[:, 0:384], lhsT=c16_sb[:, 0:O],
                                 rhs=c16_sb[:, 0:384], start=True, stop=True)

            # work items: (chunk, col_lo, col_hi); last two chunks split in
            # half to shorten the drain chain after the DMA stream ends.
            items = [(c, 0, CH) for c in range(nch - 2)]
            for c in (nch - 2, nch - 1):
                items.append((c, 0, CH // 2))
                items.append((c, CH // 2, CH))

            def fc1(it):
                c, lo, hi = it
                xt = xts[c]
                ps1 = p1pool.tile([H1, CH], F32, tag="ps1", name="ps1")[:, :hi - lo]
                last = None
                for k in range(KS):
                    w1k = c16_sb[:, C16_W1 + k * H1:C16_W1 + (k + 1) * H1]
                    nc.tensor.matmul(ps1, lhsT=w1k, rhs=xt[:, 0, k, lo:hi],
                                     start=(k == 0), stop=False)
                    last = nc.tensor.matmul(ps1, lhsT=w1k,
                                            rhs=xt[:, 1, k, lo:hi],
                                            start=False, stop=(k == KS - 1))
                return ps1, last

            def s_exp(ps5, it):
                """exp (bf16) + DVE copy of the logits to SBUF (frees ps5)."""
                n = it[2] - it[1]
                e = mids.tile([O, CH], BF16, tag="e", name="e")[:, :n]
                nc.scalar.activation(e, ps5, AF.Exp, bias=b5_c)
                lg = mids.tile([O, CH], F32, tag="lg", name="lg", bufs=4)[:, :n]
                nc.vector.tensor_copy(lg, ps5)
                return e, lg

            def s_sum(e, it):
                n = it[2] - it[1]
                psS = pSpool.tile([O, CH], F32, tag="psS", name="psS")[:, :n]
                h = nc.tensor.matmul(psS, lhsT=ones_sb, rhs=e, start=True,
                                     stop=True)
                return psS, h

            def s_fin(psS, lg, it):
                """ln -> (logits+b5)-lse -> store."""
                c, lo, hi = it
                n = hi - lo
                lse = mids.tile([O, CH], F32, tag="lse", name="lse")[:, :n]
                nc.scalar.activation(lse, psS, AF.Ln)
                oslice = olog[:, c * CH + lo:c * CH + hi]
                nc.vector.scalar_tensor_tensor(
                    out=oslice, in0=lg, scalar=b5_c, in1=lse,
                    op0=OP.add, op1=OP.subtract)
                nc.gpsimd.dma_start(out=y[:, c * CH + lo:c * CH + hi], in_=oslice)

            # software pipeline, deep lags so every cross-engine input is a
            # full iteration old (ACT's ~3us op block completes during the
            # producer's next fc1, so the PE never waits on ACT):
            #   PE:  fc1(i) fc2(i-2) fc5(i-4) sum(i-6)
            #   ACT: sign1(i-1) sign2(i-3) exp(i-5) ln(i-7)
            #   DVE: copy(i-5) stt(i-7); store(i-7) on gpsimd
            ps1s, y1s, ps2s, y2s, ps5s, es, lgs, pSs = ({} for _ in range(8))
            ni = len(items)

            def sign1(ps1, it):
                n = it[2] - it[1]
                y1 = mids.tile([H1, CH], F16, tag="y1", name="y1")[:, :n]
                nc.scalar.activation(y1, ps1, AF.Sign, bias=cs1_t, scale=cs1_s)
                return y1

            def fc2(y1, it):
                n = it[2] - it[1]
                ps2 = p2pool.tile([32 + H2, CH], F32, tag="ps2",
                                  name="ps2")[32:32 + H2, :n]
                h = nc.tensor.matmul(ps2, lhsT=w2_sb, rhs=y1, start=True,
                                     stop=True)
                return ps2, h

            def sign2(ps2, it):
                n = it[2] - it[1]
                y2 = mids.tile([32 + H2, CH], F16, tag="y2",
                               name="y2")[32:32 + H2, :n]
                nc.scalar.activation(y2, ps2, AF.Sign, bias=cs2_t, scale=cs2_s)
                return y2

            def fc5(y2, it):
                n = it[2] - it[1]
                ps5 = p5pool.tile([O, CH], F32, tag="ps5", name="ps5")[:, :n]
                h = nc.tensor.matmul(ps5, lhsT=w5_sb, rhs=y2, start=True,
                                     stop=True)
                return ps5, h

            fc1_last, fc2_h, fc5_h, sum_h = {}, {}, {}, {}
            for i in range(ni + 8):
                if i < ni:
                    ps1s[i], fc1_last[i] = fc1(items[i])
                if 0 <= i - 1 < ni:
                    y1s[i - 1] = sign1(ps1s.pop(i - 1), items[i - 1])
                if 0 <= i - 2 < ni:
                    ps2s[i - 2], fc2_h[i - 2] = fc2(y1s.pop(i - 2),
                                                    items[i - 2])
                if 0 <= i - 3 < ni:
                    y2s[i - 3] = sign2(ps2s.pop(i - 3), items[i - 3])
                if 0 <= i - 4 < ni:
                    ps5s[i - 4], fc5_h[i - 4] = fc5(y2s.pop(i - 4),
                                                    items[i - 4])
                if 0 <= i - 5 < ni:
                    es[i - 5], lgs[i - 5] = s_exp(ps5s.pop(i - 5), items[i - 5])
                if 0 <= i - 6 < ni:
                    pSs[i - 6], sum_h[i - 6] = s_sum(es.pop(i - 6),
                                                     items[i - 6])
                if 0 <= i - 7 < ni:
                    s_fin(pSs.pop(i - 7), lgs.pop(i - 7), items[i - 7])

            # Artificial PE ordering deps: the scheduler otherwise packs
            # fc2(c)/fc5(c)/sum(c) directly behind fc1(c), recreating a
            # zero-slack PE<->ACT serial chain (the PE then stalls ~1.5us per
            # chunk waiting on sign1/sign2).  Forcing them after later fc1
            # blocks gives every ACT stage a full fc1-block of slack.
            import bass_rust as _br
            _dep = _br.DependencyInfo(sync=True, no_sync=False)
            for c, h in fc2_h.items():
                if c + 1 in fc1_last:
                    h.ins.add_dependency(fc1_last[c + 1].ins.name, _dep)
            for c, h in fc5_h.items():
                if c + 2 in fc1_last:
                    h.ins.add_dependency(fc1_last[c + 2].ins.name, _dep)
            for c, h in sum_h.items():
                if c + 3 in fc1_last:
                    h.ins.add_dependency(fc1_last[c + 3].ins.name, _dep)

    nc.finalize()
    return nc


def _prep_inputs(x, w1, b1, g1, be1, m1, v1, w2, b2, g2, be2, m2, v2, w5, b5,
                 bpc: int = BPC, n_cores: int = N_CORES):
    f64 = np.float64
    w1s = np.where(w1 >= 0, 1.0, -1.0).astype(np.float32)
    w2s = np.where(w2 >= 0, 1.0, -1.0).astype(np.float32)
    w5s = np.where(w5 >= 0, 1.0, -1.0).astype(np.float32)

    w1t = np.ascontiguousarray(w1s.T)                              # [D, H1]
    w1sw = w1t.reshape(KS, 128, H1).transpose(1, 0, 2)             # [128,KS,H1]

    c16 = np.zeros((128, C16_N), dtype=np.float16)
    c16[:, C16_W1:C16_W1 + KS * H1] = w1sw.reshape(128, KS * H1)
    c16[0:H1, C16_W2:C16_W2 + H2] = w2s.T.astype(np.float16)
    c16[32:32 + H2, C16_W5:C16_W5 + O] = w5s.T.astype(np.float16)

    s1 = (g1.astype(f64) / np.sqrt(v1.astype(f64) + EPS))
    t1 = s1 * (b1.astype(f64) - m1.astype(f64)) + be1.astype(f64)
    s2 = (g2.astype(f64) / np.sqrt(v2.astype(f64) + EPS))
    t2 = s2 * (b2.astype(f64) - m2.astype(f64)) + be2.astype(f64)

    c32 = np.zeros((128, C32_N), dtype=np.float32)
    c32[0:H1, C32_CS1] = s1 / LO   # psum carries 2048*h1
    c32[0:H1, C32_CS1 + 1] = t1
    c32[32:32 + H2, C32_CS2] = s2
    c32[32:32 + H2, C32_CS2 + 1] = t2
    c32[0:O, C32_B5] = b5.astype(np.float32)

    import ml_dtypes
    cbb = np.zeros((128, CB_N), dtype=ml_dtypes.bfloat16)
    cbb[0:O, CB_ONES:CB_ONES + O] = 1.0

    x = np.asarray(x, dtype=np.float32)
    xh = x.astype(np.float16)
    xhs = (xh.astype(np.float32) * LO).astype(np.float16)  # exact scale
    xl = ((x - xh.astype(np.float32)) * LO).astype(np.float16)

    nch = bpc // CH

    def swizzle(a):  # [bpc, D] -> [128, nch, KS, CH]
        return np.ascontiguousarray(
            a.T.reshape(KS, 128, nch, CH).transpose(1, 2, 0, 3))

    in_maps = []
    for c in range(n_cores):
        rs = slice(c * bpc, (c + 1) * bpc)
        xpk = np.stack([swizzle(xhs[rs]), swizzle(xl[rs])], axis=2)
        in_maps.append({
            "xp": np.ascontiguousarray(xpk),  # [128, nch, 2, KS, CH]
            "c16": c16, "c32": c32, "cb": cbb,
        })
    return in_maps


def _decode_output(y_dev: np.ndarray, bpc: int) -> np.ndarray:
    return np.ascontiguousarray(y_dev.T)


_CACHED = {}


def kernel(**inputs) -> np.ndarray:
    from concourse.bass_utils import run_bass_kernel_spmd

    in_maps = _prep_inputs(**inputs)
    if "nc" not in _CACHED:
        _CACHED["nc"] = build_bass()
    nc = _CACHED["nc"]
    res = run_bass_kernel_spmd(nc, in_maps, list(range(N_CORES)))
    out = np.empty((B, O), dtype=np.float32)
    for c in range(N_CORES):
        out[c * BPC:(c + 1) * BPC] = _decode_output(res.results[c]["y"], BPC)
    return out


# revision 3
# speedup vs baseline: 1.1272x; 1.1272x over previous
# Trainium2 Bass kernel for a binarized 2-block MLP (BNN):
#   h1 = sign(BN1(x @ sign(w1).T + b1)); h2 = sign(BN2(h1 @ sign(w2).T + b2))
#   out = log_softmax(h2 @ sign(w5).T + b5)
#
# v4 (on top of v3's feature-major back half):
#   * hi part shipped pre-scaled by 2048 (exact in fp16), so the hi and lo
#     fc1 passes share ONE +-1 weight matrix: 8 LDWEIGHTS + 16 MMs per
#     chunk instead of 16+16. The 2^-11 folds into the BN1 scale
#     (bit-exact: pure exponent shifts).
#   * ln/stt/store lag one extra iteration so ACT never waits on the
#     same-iteration sum matmul; a DVE copy of ps5 -> SBUF keeps the PSUM
#     pools at 8 banks.
#   * ~10 dummy matmuls (gated only on the constants DMA) warm the PE HAM
#     clock to 2.4GHz before real work arrives.
#   * retained: packed consts first, chunk-granular x DMA with chunk 0 in
#     4 k-slabs, last two chunks' compute split in halves, per-chunk
#     feature-major stores.
import os
import sys

import numpy as np

for _p in ("/opt/trn_rl_repo", "/root/.axon_site/_ro/trn_rl_repo"):
    if os.path.isdir(_p) and _p not in sys.path:
        sys.path.insert(0, _p)

import concourse.bass as bass
import concourse.mybir as mybir
import concourse.tile as tile
from concourse import bacc

N_CORES = 8
B, D, H1, H2, O = 65536, 1024, 50, 20, 10
BPC = B // N_CORES  # batch rows per core
CH = 512            # batch chunk (one PSUM bank of fp32)
KS = D // 128       # contraction slices
EPS = 1e-4
LO = 2048.0         # hi-part pre-scale (2**11)

F16 = mybir.dt.float16
BF16 = mybir.dt.bfloat16
F32 = mybir.dt.float32
AF = mybir.ActivationFunctionType
AX = mybir.AxisListType
OP = mybir.AluOpType

# packed fp16 consts layout (columns)
C16_W1 = 0                  # [128, KS*H1]  sign(w1) swizzled
C16_W2 = KS * H1            # rows 0:50, [50, H2]
C16_W5 = C16_W2 + H2        # rows 0:20, [20, O]
C16_N = C16_W5 + O
# packed fp32 consts layout
C32_CS1 = 0                 # rows 0:50, [50, 2]  (scale/2048, shift)
C32_CS2 = 2                 # rows 0:20, [20, 2]
C32_B5 = 4                  # rows 0:10, [10, 1]
C32_N = 5
# packed bf16 consts
CB_ONES = 0                 # rows 0:10, [10, 10] all-ones
CB_N = O


def build_bass(bpc: int = BPC) -> bass.Bass:
    nch = bpc // CH
    nc = bacc.Bacc("TRN2", target_bir_lowering=False)

    # Restrict the ACT-table chooser to the combined set so Sign/Exp/Ln all
    # come from one table load.
    def _act_table_loads_combined_set_only(self=nc):
        import bass_rust as _br

        from concourse.hw_specs import get_activation_tables

        has_act = any(
            isinstance(i, mybir.InstActivation)
            for blk in self.main_func.blocks
            for i in blk.instructions
        )
        if not has_act:
            return
        tabs = get_activation_tables(self.m.arch)
        tables = [
            (name, fns if name == "natural_log_exp_and_others" else set())
            for name, fns in tabs.items()
        ]
        _br.insert_act_table_loads(self, tables)

    nc.insert_act_table_loads = _act_table_loads_combined_set_only

    # x arrives packed: xp[p, c, h, k, n] = part h (0=hi*2048, 1=lo) of
    # x.T[k*128+p, c*CH+n]; per partition a chunk slice is contiguous 16KB.
    xp = nc.declare_dram_parameter("xp", [128, nch, 2, KS, CH], F16, isOutput=False)
    c16 = nc.declare_dram_parameter("c16", [128, C16_N], F16, isOutput=False)
    c32 = nc.declare_dram_parameter("c32", [128, C32_N], F32, isOutput=False)
    cb = nc.declare_dram_parameter("cb", [128, CB_N], BF16, isOutput=False)
    # Output, feature-major: y[o, r] = out[r, o]
    y = nc.declare_dram_parameter("y", [O, bpc], F32, isOutput=True)

    with tile.TileContext(nc) as tc:
        from contextlib import ExitStack

        with ExitStack() as ctx:
            singles = ctx.enter_context(tc.tile_pool(name="singles", bufs=1))
            xpool = ctx.enter_context(tc.tile_pool(name="xpool", bufs=8))
            mids = ctx.enter_context(tc.tile_pool(name="mids", bufs=3))
            p1pool = ctx.enter_context(tc.tile_pool(name="p1", bufs=2, space="PSUM"))
            p2pool = ctx.enter_context(tc.tile_pool(name="p2", bufs=2, space="PSUM"))
            p5pool = ctx.enter_context(tc.tile_pool(name="p5", bufs=2, space="PSUM"))
            pSpool = ctx.enter_context(tc.tile_pool(name="pS", bufs=2, space="PSUM"))

            # fc1 consts first, then the head of the x stream (chunk 0 as
            # 4 k-slabs so the first matmuls begin earlier), then the
            # remaining consts, then the rest of the stream.
            c16_sb = singles.tile([128, C16_N], F16)
            nc.sync.dma_start(out=c16_sb, in_=c16[:, :])
            c32_sb = singles.tile([128, C32_N], F32)
            cb_sb = singles.tile([128, CB_N], BF16)

            xts = []
            for c in range(nch):
                xt = xpool.tile([128, 2, KS, CH], F16, tag="x", name="x_t")
                nslab = {0: 4}.get(c, 1)
                ks_per = KS // nslab
                for s in range(nslab):
                    sl = slice(s * ks_per, (s + 1) * ks_per)
                    nc.sync.dma_start(out=xt[:, :, sl, :],
                                      in_=xp[:, c, :, sl, :])
                xts.append(xt)
                if c == 2:
                    nc.sync.dma_start(out=c32_sb, in_=c32[:, :])
                    nc.sync.dma_start(out=cb_sb, in_=cb[:, :])

            cs1_s = c32_sb[0:H1, C32_CS1:C32_CS1 + 1]
            cs1_t = c32_sb[0:H1, C32_CS1 + 1:C32_CS1 + 2]
            cs2_s = c32_sb[32:32 + H2, C32_CS2:C32_CS2 + 1]
            cs2_t = c32_sb[32:32 + H2, C32_CS2 + 1:C32_CS2 + 2]
            b5_c = c32_sb[0:O, C32_B5:C32_B5 + 1]
            w2_sb = c16_sb[0:H1, C16_W2:C16_W2 + H2]
            w5_sb = c16_sb[32:32 + H2, C16_W5:C16_W5 + O]
            ones_sb = cb_sb[0:O, CB_ONES:CB_ONES + O]

            olog = singles.tile([O, bpc], F32)

            # PE warmup: dummy matmuls gated only on the consts DMA keep the
            # HAM activity window busy while the first x slabs land, so real
            # work starts at 2.4GHz. Output goes to the pS ring (never read).
            warm = pSpool.tile([O, CH], F32, tag="psS", name="warm")
            for _ in range(10):
                nc.tensor.matmul(warm[:, 0:384], lhsT=c16_sb[:, 0:O],
                                 rhs=c16_sb[:, 0:384], start=True, stop=True)

            # work items: (chunk, col_lo, col_hi); last two chunks split in
            # half to shorten the drain chain after the DMA stream ends.
            items = [(c, 0, CH) for c in range(nch - 2)]
            for c in (nch - 2, nch - 1):
                items.append((c, 0, CH // 2))
                items.append((c, CH // 2, CH))

            def fc1(it):
                c, lo, hi = it
                xt = xts[c]
                ps1 = p1pool.tile([H1, CH], F32, tag="ps1", name="ps1")[:, :hi - lo]
                last = None
                for k in range(KS):
                    w1k = c16_sb[:, C16_W1 + k * H1:C16_W1 + (k + 1) * H1]
                    nc.tensor.matmul(ps1, lhsT=w1k, rhs=xt[:, 0, k, lo:hi],
                                     start=(k == 0), stop=False)
                    last = nc.tensor.matmul(ps1, lhsT=w1k,
                                            rhs=xt[:, 1, k, lo:hi],
                                            start=False, stop=(k == KS - 1))
                return ps1, last

            def s_exp(ps5, it):
                """exp (bf16) + DVE copy of the logits to SBUF (frees ps5)."""
                n = it[2] - it[1]
                e = mids.tile([O, CH], BF16, tag="e", name="e")[:, :n]
                nc.scalar.activation(e, ps5, AF.Exp, bias=b5_c)
                lg = mids.tile([O, CH], F32, tag="lg", name="lg", bufs=4)[:, :n]
                nc.vector.tensor_copy(lg, ps5)
                return e, lg

            def s_sum(e, it):
                n = it[2] - it[1]
                psS = pSpool.tile([O, CH], F32, tag="psS", name="psS")[:, :n]
                h = nc.tensor.matmul(psS, lhsT=ones_sb, rhs=e, start=True,
                                     stop=True)
                return psS, h

            def s_fin(psS, lg, it):
                """ln -> (logits+b5)-lse -> store."""
                c, lo, hi = it
                n = hi - lo
                lse = mids.tile([O, CH], F32, tag="lse", name="lse")[:, :n]
                nc.scalar.activation(lse, psS, AF.Ln)
                oslice = olog[:, c * CH + lo:c * CH + hi]
                nc.vector.scalar_tensor_tensor(
                    out=oslice, in0=lg, scalar=b5_c, in1=lse,
                    op0=OP.add, op1=OP.subtract)
                if hi == CH:  # store once per chunk (SWDGE keeps the sync
                    # HWDGE ring free for the x stream)
                    nc.gpsimd.dma_start(out=y[:, c * CH:(c + 1) * CH],
                                        in_=olog[:, c * CH:(c + 1) * CH])

            # software pipeline, deep lags so every cross-engine input is a
            # full iteration old (ACT's ~3us op block completes during the
            # producer's next fc1, so the PE never waits on ACT):
            #   PE:  fc1(i) fc2(i-2) fc5(i-4) sum(i-6)
            #   ACT: sign1(i-1) sign2(i-3) exp(i-5) ln(i-7)
            #   DVE: copy(i-5) stt(i-7); store(i-7) on gpsimd
            ps1s, y1s, ps2s, y2s, ps5s, es, lgs, pSs = ({} for _ in range(8))
            ni = len(items)

            def sign1(ps1, it):
                n = it[2] - it[1]
                y1 = mids.tile([H1, CH], F16, tag="y1", name="y1")[:, :n]
                nc.scalar.activation(y1, ps1, AF.Sign, bias=cs1_t, scale=cs1_s)
                return y1

            def fc2(y1, it):
                n = it[2] - it[1]
                ps2 = p2pool.tile([32 + H2, CH], F32, tag="ps2",
                                  name="ps2")[32:32 + H2, :n]
                h = nc.tensor.matmul(ps2, lhsT=w2_sb, rhs=y1, start=True,
                                     stop=True)
                return ps2, h

            def sign2(ps2, it):
                n = it[2] - it[1]
                y2 = mids.tile([32 + H2, CH], F16, tag="y2",
                               name="y2")[32:32 + H2, :n]
                nc.scalar.activation(y2, ps2, AF.Sign, bias=cs2_t, scale=cs2_s)
                return y2

            def fc5(y2, it):
                n = it[2] - it[1]
                ps5 = p5pool.tile([O, CH], F32, tag="ps5", name="ps5")[:, :n]
                h = nc.tensor.matmul(ps5, lhsT=w5_sb, rhs=y2, start=True,
                                     stop=True)
                return ps5, h

            fc1_last, fc2_h, fc5_h, sum_h = {}, {}, {}, {}
            for i in range(ni + 8):
                if i < ni:
                    ps1s[i], fc1_last[i] = fc1(items[i])
                if 0 <= i - 1 < ni:
                    y1s[i - 1] = sign1(ps1s.pop(i - 1), items[i - 1])
                if 0 <= i - 2 < ni:
                    ps2s[i - 2], fc2_h[i - 2] = fc2(y1s.pop(i - 2),
                                                    items[i - 2])
                if 0 <= i - 3 < ni:
                    y2s[i - 3] = sign2(ps2s.pop(i - 3), items[i - 3])
                if 0 <= i - 4 < ni:
                    ps5s[i - 4], fc5_h[i - 4] = fc5(y2s.pop(i - 4),
                                                    items[i - 4])
                if 0 <= i - 5 < ni:
                    es[i - 5], lgs[i - 5] = s_exp(ps5s.pop(i - 5), items[i - 5])
                if 0 <= i - 6 < ni:
                    pSs[i - 6], sum_h[i - 6] = s_sum(es.pop(i - 6),
                                                     items[i - 6])
                if 0 <= i - 7 < ni:
                    s_fin(pSs.pop(i - 7), lgs.pop(i - 7), items[i - 7])

            # Artificial PE ordering deps: the scheduler otherwise packs
            # fc2(c)/fc5(c)/sum(c) directly behind fc1(c), recreating a
            # zero-slack PE<->ACT serial chain (the PE then stalls ~1.5us per
            # chunk waiting on sign1/sign2).  Forcing them after later fc1
            # blocks gives every ACT stage a full fc1-block of slack.
            import bass_rust as _br
            _dep = _br.DependencyInfo(sync=True, no_sync=False)
            for c, h in fc2_h.items():
                if c + 1 in fc1_last:
                    h.ins.add_dependency(fc1_last[c + 1].ins.name, _dep)
            for c, h in fc5_h.items():
                if c + 2 in fc1_last:
                    h.ins.add_dependency(fc1_last[c + 2].ins.name, _dep)
            for c, h in sum_h.items():
                if c + 3 in fc1_last:
                    h.ins.add_dependency(fc1_last[c + 3].ins.name, _dep)

    nc.finalize()
    return nc


def _prep_inputs(x, w1, b1, g1, be1, m1, v1, w2, b2, g2, be2, m2, v2, w5, b5,
                 bpc: int = BPC, n_cores: int = N_CORES):
    f64 = np.float64
    w1s = np.where(w1 >= 0, 1.0, -1.0).astype(np.float32)
    w2s = np.where(w2 >= 0, 1.0, -1.0).astype(np.float32)
    w5s = np.where(w5 >= 0, 1.0, -1.0).astype(np.float32)

    w1t = np.ascontiguousarray(w1s.T)                              # [D, H1]
    w1sw = w1t.reshape(KS, 128, H1).transpose(1, 0, 2)             # [128,KS,H1]

    c16 = np.zeros((128, C16_N), dtype=np.float16)
    c16[:, C16_W1:C16_W1 + KS * H1] = w1sw.reshape(128, KS * H1)
    c16[0:H1, C16_W2:C16_W2 + H2] = w2s.T.astype(np.float16)
    c16[32:32 + H2, C16_W5:C16_W5 + O] = w5s.T.astype(np.float16)

    s1 = (g1.astype(f64) / np.sqrt(v1.astype(f64) + EPS))
    t1 = s1 * (b1.astype(f64) - m1.astype(f64)) + be1.astype(f64)
    s2 = (g2.astype(f64) / np.sqrt(v2.astype(f64) + EPS))
    t2 = s2 * (b2.astype(f64) - m2.astype(f64)) + be2.astype(f64)

    c32 = np.zeros((128, C32_N), dtype=np.float32)
    c32[0:H1, C32_CS1] = s1 / LO   # psum carries 2048*h1
    c32[0:H1, C32_CS1 + 1] = t1
    c32[32:32 + H2, C32_CS2] = s2
    c32[32:32 + H2, C32_CS2 + 1] = t2
    c32[0:O, C32_B5] = b5.astype(np.float32)

    import ml_dtypes
    cbb = np.zeros((128, CB_N), dtype=ml_dtypes.bfloat16)
    cbb[0:O, CB_ONES:CB_ONES + O] = 1.0

    x = np.asarray(x, dtype=np.float32)
    xh = x.astype(np.float16)
    xhs = (xh.astype(np.float32) * LO).astype(np.float16)  # exact scale
    xl = ((x - xh.astype(np.float32)) * LO).astype(np.float16)

    nch = bpc // CH

    def swizzle(a):  # [bpc, D] -> [128, nch, KS, CH]
        return np.ascontiguousarray(
            a.T.reshape(KS, 128, nch, CH).transpose(1, 2, 0, 3))

    in_maps = []
    for c in range(n_cores):
        rs = slice(c * bpc, (c + 1) * bpc)
        xpk = np.stack([swizzle(xhs[rs]), swizzle(xl[rs])], axis=2)
        in_maps.append({
            "xp": np.ascontiguousarray(xpk),  # [128, nch, 2, KS, CH]
            "c16": c16, "c32": c32, "cb": cbb,
        })
    return in_maps


def _decode_output(y_dev: np.ndarray, bpc: int) -> np.ndarray:
    return np.ascontiguousarray(y_dev.T)


_CACHED = {}


def kernel(**inputs) -> np.ndarray:
    from concourse.bass_utils import run_bass_kernel_spmd

    in_maps = _prep_inputs(**inputs)
    if "nc" not in _CACHED:
        _CACHED["nc"] = build_bass()
    nc = _CACHED["nc"]
    res = run_bass_kernel_spmd(nc, in_maps, list(range(N_CORES)))
    out = np.empty((B, O), dtype=np.float32)
    for c in range(N_CORES):
        out[c * BPC:(c + 1) * BPC] = _decode_output(res.results[c]["y"], BPC)
    return out


# revision 4
# speedup vs baseline: 1.1420x; 1.0131x over previous
# Trainium2 Bass kernel for a binarized 2-block MLP (BNN):
#   h1 = sign(BN1(x @ sign(w1).T + b1)); h2 = sign(BN2(h1 @ sign(w2).T + b2))
#   out = log_softmax(h2 @ sign(w5).T + b5)
#
# v4 (on top of v3's feature-major back half):
#   * hi part shipped pre-scaled by 2048 (exact in fp16), so the hi and lo
#     fc1 passes share ONE +-1 weight matrix: 8 LDWEIGHTS + 16 MMs per
#     chunk instead of 16+16. The 2^-11 folds into the BN1 scale
#     (bit-exact: pure exponent shifts).
#   * ln/stt/store lag one extra iteration so ACT never waits on the
#     same-iteration sum matmul; a DVE copy of ps5 -> SBUF keeps the PSUM
#     pools at 8 banks.
#   * ~10 dummy matmuls (gated only on the constants DMA) warm the PE HAM
#     clock to 2.4GHz before real work arrives.
#   * retained: packed consts first, chunk-granular x DMA with chunk 0 in
#     4 k-slabs, last two chunks' compute split in halves, per-chunk
#     feature-major stores.
import os
import sys

import numpy as np

for _p in ("/opt/trn_rl_repo", "/root/.axon_site/_ro/trn_rl_repo"):
    if os.path.isdir(_p) and _p not in sys.path:
        sys.path.insert(0, _p)

import concourse.bass as bass
import concourse.mybir as mybir
import concourse.tile as tile
from concourse import bacc

N_CORES = 8
B, D, H1, H2, O = 65536, 1024, 50, 20, 10
BPC = B // N_CORES  # batch rows per core
CH = 512            # batch chunk (one PSUM bank of fp32)
KS = D // 128       # contraction slices
EPS = 1e-4
LO = 2048.0         # hi-part pre-scale (2**11)

F16 = mybir.dt.float16
BF16 = mybir.dt.bfloat16
F32 = mybir.dt.float32
AF = mybir.ActivationFunctionType
AX = mybir.AxisListType
OP = mybir.AluOpType

# packed fp16 consts layout (columns)
C16_W1 = 0                  # [128, KS*H1]  sign(w1) swizzled
C16_W2 = KS * H1            # rows 0:50, [50, H2]
C16_W5 = C16_W2 + H2        # rows 0:20, [20, O]
C16_N = C16_W5 + O
# packed fp32 consts layout
C32_CS1 = 0                 # rows 0:50, [50, 2]  (scale/2048, shift)
C32_CS2 = 2                 # rows 0:20, [20, 2]
C32_B5 = 4                  # rows 0:10, [10, 1]
C32_N = 5
# packed bf16 consts
CB_ONES = 0                 # rows 0:10, [10, 10] all-ones
CB_N = O


def build_bass(bpc: int = BPC) -> bass.Bass:
    nch = bpc // CH
    nc = bacc.Bacc("TRN2", target_bir_lowering=False)

    # Restrict the ACT-table chooser to the combined set so Sign/Exp/Ln all
    # come from one table load.
    def _act_table_loads_combined_set_only(self=nc):
        import bass_rust as _br

        from concourse.hw_specs import get_activation_tables

        has_act = any(
            isinstance(i, mybir.InstActivation)
            for blk in self.main_func.blocks
            for i in blk.instructions
        )
        if not has_act:
            return
        tabs = get_activation_tables(self.m.arch)
        tables = [
            (name, fns if name == "natural_log_exp_and_others" else set())
            for name, fns in tabs.items()
        ]
        _br.insert_act_table_loads(self, tables)

    nc.insert_act_table_loads = _act_table_loads_combined_set_only

    # x arrives packed: xp[p, c, h, k, n] = part h (0=hi*2048, 1=lo) of
    # x.T[k*128+p, c*CH+n]; per partition a chunk slice is contiguous 16KB.
    xp = nc.declare_dram_parameter("xp", [128, nch, 2, KS, CH], F16, isOutput=False)
    c16 = nc.declare_dram_parameter("c16", [128, C16_N], F16, isOutput=False)
    c32 = nc.declare_dram_parameter("c32", [128, C32_N], F32, isOutput=False)
    cb = nc.declare_dram_parameter("cb", [128, CB_N], BF16, isOutput=False)
    # Output, feature-major: y[o, r] = out[r, o]
    y = nc.declare_dram_parameter("y", [O, bpc], F32, isOutput=True)

    with tile.TileContext(nc) as tc:
        from contextlib import ExitStack

        with ExitStack() as ctx:
            singles = ctx.enter_context(tc.tile_pool(name="singles", bufs=1))
            xpool = ctx.enter_context(tc.tile_pool(name="xpool", bufs=8))
            mids = ctx.enter_context(tc.tile_pool(name="mids", bufs=3))
            p1pool = ctx.enter_context(tc.tile_pool(name="p1", bufs=2, space="PSUM"))
            p2pool = ctx.enter_context(tc.tile_pool(name="p2", bufs=2, space="PSUM"))
            p5pool = ctx.enter_context(tc.tile_pool(name="p5", bufs=2, space="PSUM"))
            pSpool = ctx.enter_context(tc.tile_pool(name="pS", bufs=2, space="PSUM"))

            # fc1 consts first, then the head of the x stream (chunk 0 as
            # 4 k-slabs so the first matmuls begin earlier), then the
            # remaining consts, then the rest of the stream.
            c16_sb = singles.tile([128, C16_N], F16)
            nc.sync.dma_start(out=c16_sb, in_=c16[:, :])
            c32_sb = singles.tile([128, C32_N], F32)
            cb_sb = singles.tile([128, CB_N], BF16)

            xts = []
            for c in range(nch):
                xt = xpool.tile([128, 2, KS, CH], F16, tag="x", name="x_t")
                nslab = {0: 4}.get(c, 1)
                ks_per = KS // nslab
                for s in range(nslab):
                    sl = slice(s * ks_per, (s + 1) * ks_per)
                    nc.sync.dma_start(out=xt[:, :, sl, :],
                                      in_=xp[:, c, :, sl, :])
                xts.append(xt)
                if c == 0:
                    nc.sync.dma_start(out=c32_sb, in_=c32[:, :])
                if c == 4:
                    nc.sync.dma_start(out=cb_sb, in_=cb[:, :])

            cs1_s = c32_sb[0:H1, C32_CS1:C32_CS1 + 1]
            cs1_t = c32_sb[0:H1, C32_CS1 + 1:C32_CS1 + 2]
            cs2_s = c32_sb[32:32 + H2, C32_CS2:C32_CS2 + 1]
            cs2_t = c32_sb[32:32 + H2, C32_CS2 + 1:C32_CS2 + 2]
            b5_c = c32_sb[0:O, C32_B5:C32_B5 + 1]
            w2_sb = c16_sb[0:H1, C16_W2:C16_W2 + H2]
            w5_sb = c16_sb[32:32 + H2, C16_W5:C16_W5 + O]
            ones_sb = cb_sb[0:O, CB_ONES:CB_ONES + O]

            olog = singles.tile([O, bpc], F32)

            # PE warmup: dummy matmuls gated only on the consts DMA keep the
            # HAM activity window busy while the first x slabs land, so real
            # work starts at 2.4GHz. Output goes to the pS ring (never read).
            warm = pSpool.tile([O, CH], F32, tag="psS", name="warm")
            for _ in range(10):
                nc.tensor.matmul(warm[:, 0:384], lhsT=c16_sb[:, 0:O],
                                 rhs=c16_sb[:, 0:384], start=True, stop=True)

            # work items: (chunk, col_lo, col_hi); last two chunks split in
            # half to shorten the drain chain after the DMA stream ends.
            items = [(c, 0, CH) for c in range(nch - 2)]
            for c in (nch - 2, nch - 1):
                items.append((c, 0, CH // 2))
                items.append((c, CH // 2, CH))

            def fc1(it):
                c, lo, hi = it
                xt = xts[c]
                ps1 = p1pool.tile([H1, CH], F32, tag="ps1", name="ps1")[:, :hi - lo]
                last = None
                for k in range(KS):
                    w1k = c16_sb[:, C16_W1 + k * H1:C16_W1 + (k + 1) * H1]
                    nc.tensor.matmul(ps1, lhsT=w1k, rhs=xt[:, 0, k, lo:hi],
                                     start=(k == 0), stop=False)
                    last = nc.tensor.matmul(ps1, lhsT=w1k,
                                            rhs=xt[:, 1, k, lo:hi],
                                            start=False, stop=(k == KS - 1))
                return ps1, last

            def s_exp(ps5, it):
                """exp (bf16) + DVE copy of the logits to SBUF (frees ps5)."""
                n = it[2] - it[1]
                e = mids.tile([O, CH], BF16, tag="e", name="e")[:, :n]
                nc.scalar.activation(e, ps5, AF.Exp, bias=b5_c)
                lg = mids.tile([O, CH], F32, tag="lg", name="lg", bufs=4)[:, :n]
                nc.vector.tensor_copy(lg, ps5)
                return e, lg

            def s_sum(e, it):
                n = it[2] - it[1]
                psS = pSpool.tile([O, CH], F32, tag="psS", name="psS")[:, :n]
                h = nc.tensor.matmul(psS, lhsT=ones_sb, rhs=e, start=True,
                                     stop=True)
                return psS, h

            def s_fin(psS, lg, it):
                """ln -> (logits+b5)-lse -> store."""
                c, lo, hi = it
                n = hi - lo
                lse = mids.tile([O, CH], F32, tag="lse", name="lse")[:, :n]
                nc.scalar.activation(lse, psS, AF.Ln)
                oslice = olog[:, c * CH + lo:c * CH + hi]
                nc.vector.scalar_tensor_tensor(
                    out=oslice, in0=lg, scalar=b5_c, in1=lse,
                    op0=OP.add, op1=OP.subtract)
                if hi == CH:  # store once per chunk (SWDGE keeps the sync
                    # HWDGE ring free for the x stream)
                    nc.gpsimd.dma_start(out=y[:, c * CH:(c + 1) * CH],
                                        in_=olog[:, c * CH:(c + 1) * CH])

            # software pipeline, deep lags so every cross-engine input is a
            # full iteration old (ACT's ~3us op block completes during the
            # producer's next fc1, so the PE never waits on ACT):
            #   PE:  fc1(i) fc2(i-2) fc5(i-4) sum(i-6)
            #   ACT: sign1(i-1) sign2(i-3) exp(i-5) ln(i-7)
            #   DVE: copy(i-5) stt(i-7); store(i-7) on gpsimd
            ps1s, y1s, ps2s, y2s, ps5s, es, lgs, pSs = ({} for _ in range(8))
            ni = len(items)

            def sign1(ps1, it):
                n = it[2] - it[1]
                y1 = mids.tile([H1, CH], F16, tag="y1", name="y1")[:, :n]
                nc.scalar.activation(y1, ps1, AF.Sign, bias=cs1_t, scale=cs1_s)
                return y1

            def fc2(y1, it):
                n = it[2] - it[1]
                ps2 = p2pool.tile([32 + H2, CH], F32, tag="ps2",
                                  name="ps2")[32:32 + H2, :n]
                h = nc.tensor.matmul(ps2, lhsT=w2_sb, rhs=y1, start=True,
                                     stop=True)
                return ps2, h

            def sign2(ps2, it):
                n = it[2] - it[1]
                y2 = mids.tile([32 + H2, CH], F16, tag="y2",
                               name="y2")[32:32 + H2, :n]
                nc.scalar.activation(y2, ps2, AF.Sign, bias=cs2_t, scale=cs2_s)
                return y2

            def fc5(y2, it):
                n = it[2] - it[1]
                ps5 = p5pool.tile([O, CH], F32, tag="ps5", name="ps5")[:, :n]
                h = nc.tensor.matmul(ps5, lhsT=w5_sb, rhs=y2, start=True,
                                     stop=True)
                return ps5, h

            fc1_last, fc2_h, fc5_h, sum_h = {}, {}, {}, {}
            for i in range(ni + 8):
                if i < ni:
                    ps1s[i], fc1_last[i] = fc1(items[i])
                if 0 <= i - 1 < ni:
                    y1s[i - 1] = sign1(ps1s.pop(i - 1), items[i - 1])
                if 0 <= i - 2 < ni:
                    ps2s[i - 2], fc2_h[i - 2] = fc2(y1s.pop(i - 2),
                                                    items[i - 2])
                if 0 <= i - 3 < ni:
                    y2s[i - 3] = sign2(ps2s.pop(i - 3), items[i - 3])
                if 0 <= i - 4 < ni:
                    ps5s[i - 4], fc5_h[i - 4] = fc5(y2s.pop(i - 4),
                                                    items[i - 4])
                if 0 <= i - 5 < ni:
                    es[i - 5], lgs[i - 5] = s_exp(ps5s.pop(i - 5), items[i - 5])
                if 0 <= i - 6 < ni:
                    pSs[i - 6], sum_h[i - 6] = s_sum(es.pop(i - 6),
                                                     items[i - 6])
                if 0 <= i - 7 < ni:
                    s_fin(pSs.pop(i - 7), lgs.pop(i - 7), items[i - 7])
                if i >= ni:
                    # dummy weight loads keep the PE HAM activity window
                    # busy through the flush ping-pong (no PSUM writes, no
                    # consumers -- correctness-neutral)
                    for _ in range(4):
                        nc.tensor.ldweights(weights=c16_sb[:, 0:H1])

            # Artificial PE ordering deps: the scheduler otherwise packs
            # fc2(c)/fc5(c)/sum(c) directly behind fc1(c), recreating a
            # zero-slack PE<->ACT serial chain (the PE then stalls ~1.5us per
            # chunk waiting on sign1/sign2).  Forcing them after later fc1
            # blocks gives every ACT stage a full fc1-block of slack.
            import bass_rust as _br
            _dep = _br.DependencyInfo(sync=True, no_sync=False)
            for c, h in fc2_h.items():
                if c + 1 in fc1_last:
                    h.ins.add_dependency(fc1_last[c + 1].ins.name, _dep)
            for c, h in fc5_h.items():
                if c + 2 in fc1_last:
                    h.ins.add_dependency(fc1_last[c + 2].ins.name, _dep)
            for c, h in sum_h.items():
                if c + 3 in fc1_last:
                    h.ins.add_dependency(fc1_last[c + 3].ins.name, _dep)

    nc.finalize()
    return nc


def _prep_inputs(x, w1, b1, g1, be1, m1, v1, w2, b2, g2, be2, m2, v2, w5, b5,
                 bpc: int = BPC, n_cores: int = N_CORES):
    f64 = np.float64
    w1s = np.where(w1 >= 0, 1.0, -1.0).astype(np.float32)
    w2s = np.where(w2 >= 0, 1.0, -1.0).astype(np.float32)
    w5s = np.where(w5 >= 0, 1.0, -1.0).astype(np.float32)

    w1t = np.ascontiguousarray(w1s.T)                              # [D, H1]
    w1sw = w1t.reshape(KS, 128, H1).transpose(1, 0, 2)             # [128,KS,H1]

    c16 = np.zeros((128, C16_N), dtype=np.float16)
    c16[:, C16_W1:C16_W1 + KS * H1] = w1sw.reshape(128, KS * H1)
    c16[0:H1, C16_W2:C16_W2 + H2] = w2s.T.astype(np.float16)
    c16[32:32 + H2, C16_W5:C16_W5 + O] = w5s.T.astype(np.float16)

    s1 = (g1.astype(f64) / np.sqrt(v1.astype(f64) + EPS))
    t1 = s1 * (b1.astype(f64) - m1.astype(f64)) + be1.astype(f64)
    s2 = (g2.astype(f64) / np.sqrt(v2.astype(f64) + EPS))
    t2 = s2 * (b2.astype(f64) - m2.astype(f64)) + be2.astype(f64)

    c32 = np.zeros((128, C32_N), dtype=np.float32)
    c32[0:H1, C32_CS1] = s1 / LO   # psum carries 2048*h1
    c32[0:H1, C32_CS1 + 1] = t1
    c32[32:32 + H2, C32_CS2] = s2
    c32[32:32 + H2, C32_CS2 + 1] = t2
    c32[0:O, C32_B5] = b5.astype(np.float32)

    import ml_dtypes
    cbb = np.zeros((128, CB_N), dtype=ml_dtypes.bfloat16)
    cbb[0:O, CB_ONES:CB_ONES + O] = 1.0

    x = np.asarray(x, dtype=np.float32)
    xh = x.astype(np.float16)
    xhs = (xh.astype(np.float32) * LO).astype(np.float16)  # exact scale
    xl = ((x - xh.astype(np.float32)) * LO).astype(np.float16)

    nch = bpc // CH

    def swizzle(a):  # [bpc, D] -> [128, nch, KS, CH]
        return np.ascontiguousarray(
            a.T.reshape(KS, 128, nch, CH).transpose(1, 2, 0, 3))

    in_maps = []
    for c in range(n_cores):
        rs = slice(c * bpc, (c + 1) * bpc)
        xpk = np.stack([swizzle(xhs[rs]), swizzle(xl[rs])], axis=2)
        in_maps.append({
            "xp": np.ascontiguousarray(xpk),  # [128, nch, 2, KS, CH]
            "c16": c16, "c32": c32, "cb": cbb,
        })
    return in_maps


def _decode_output(y_dev: np.ndarray, bpc: int) -> np.ndarray:
    return np.ascontiguousarray(y_dev.T)


_CACHED = {}


def kernel(**inputs) -> np.ndarray:
    from concourse.bass_utils import run_bass_kernel_spmd

    in_maps = _prep_inputs(**inputs)
    if "nc" not in _CACHED:
        _CACHED["nc"] = build_bass()
    nc = _CACHED["nc"]
    res = run_bass_kernel_spmd(nc, in_maps, list(range(N_CORES)))
    out = np.empty((B, O), dtype=np.float32)
    for c in range(N_CORES):
        out[c * BPC:(c + 1) * BPC] = _decode_output(res.results[c]["y"], BPC)
    return out


# revision 5
# speedup vs baseline: 1.1448x; 1.0025x over previous
# Trainium2 Bass kernel for a binarized 2-block MLP (BNN):
#   h1 = sign(BN1(x @ sign(w1).T + b1)); h2 = sign(BN2(h1 @ sign(w2).T + b2))
#   out = log_softmax(h2 @ sign(w5).T + b5)
#
# v4 (on top of v3's feature-major back half):
#   * hi part shipped pre-scaled by 2048 (exact in fp16), so the hi and lo
#     fc1 passes share ONE +-1 weight matrix: 8 LDWEIGHTS + 16 MMs per
#     chunk instead of 16+16. The 2^-11 folds into the BN1 scale
#     (bit-exact: pure exponent shifts).
#   * ln/stt/store lag one extra iteration so ACT never waits on the
#     same-iteration sum matmul; a DVE copy of ps5 -> SBUF keeps the PSUM
#     pools at 8 banks.
#   * ~10 dummy matmuls (gated only on the constants DMA) warm the PE HAM
#     clock to 2.4GHz before real work arrives.
#   * retained: packed consts first, chunk-granular x DMA with chunk 0 in
#     4 k-slabs, last two chunks' compute split in halves, per-chunk
#     feature-major stores.
import os
import sys

import numpy as np

for _p in ("/opt/trn_rl_repo", "/root/.axon_site/_ro/trn_rl_repo"):
    if os.path.isdir(_p) and _p not in sys.path:
        sys.path.insert(0, _p)

import concourse.bass as bass
import concourse.mybir as mybir
import concourse.tile as tile
from concourse import bacc

N_CORES = 8
B, D, H1, H2, O = 65536, 1024, 50, 20, 10
BPC = B // N_CORES  # batch rows per core
CH = 512            # batch chunk (one PSUM bank of fp32)
KS = D // 128       # contraction slices
EPS = 1e-4
LO = 2048.0         # hi-part pre-scale (2**11)

F16 = mybir.dt.float16
BF16 = mybir.dt.bfloat16
F32 = mybir.dt.float32
AF = mybir.ActivationFunctionType
AX = mybir.AxisListType
OP = mybir.AluOpType

# packed fp16 consts layout (columns)
C16_W1 = 0                  # [128, KS*H1]  sign(w1) swizzled
C16_W2 = KS * H1            # rows 0:50, [50, H2]
C16_W5 = C16_W2 + H2        # rows 0:20, [20, O]
C16_N = C16_W5 + O
# packed fp32 consts layout
C32_CS1 = 0                 # rows 0:50, [50, 2]  (scale/2048, shift)
C32_CS2 = 2                 # rows 0:20, [20, 2]
C32_B5 = 4                  # rows 0:10, [10, 1]
C32_N = 5
# packed bf16 consts
CB_ONES = 0                 # rows 0:10, [10, 10] all-ones
CB_N = O


def build_bass(bpc: int = BPC) -> bass.Bass:
    nch = bpc // CH
    nc = bacc.Bacc("TRN2", target_bir_lowering=False)

    # Restrict the ACT-table chooser to the combined set so Sign/Exp/Ln all
    # come from one table load.
    def _act_table_loads_combined_set_only(self=nc):
        import bass_rust as _br

        from concourse.hw_specs import get_activation_tables

        has_act = any(
            isinstance(i, mybir.InstActivation)
            for blk in self.main_func.blocks
            for i in blk.instructions
        )
        if not has_act:
            return
        tabs = get_activation_tables(self.m.arch)
        tables = [
            (name, fns if name == "natural_log_exp_and_others" else set())
            for name, fns in tabs.items()
        ]
        _br.insert_act_table_loads(self, tables)

    nc.insert_act_table_loads = _act_table_loads_combined_set_only

    # x arrives packed: xp[p, c, h, k, n] = part h (0=hi*2048, 1=lo) of
    # x.T[k*128+p, c*CH+n]; per partition a chunk slice is contiguous 16KB.
    xp = nc.declare_dram_parameter("xp", [128, nch, 2, KS, CH], F16, isOutput=False)
    c16 = nc.declare_dram_parameter("c16", [128, C16_N], F16, isOutput=False)
    c32 = nc.declare_dram_parameter("c32", [128, C32_N], F32, isOutput=False)
    cb = nc.declare_dram_parameter("cb", [128, CB_N], BF16, isOutput=False)
    # Output, feature-major: y[o, r] = out[r, o]
    y = nc.declare_dram_parameter("y", [O, bpc], F32, isOutput=True)

    with tile.TileContext(nc) as tc:
        from contextlib import ExitStack

        with ExitStack() as ctx:
            singles = ctx.enter_context(tc.tile_pool(name="singles", bufs=1))
            xpool = ctx.enter_context(tc.tile_pool(name="xpool", bufs=8))
            mids = ctx.enter_context(tc.tile_pool(name="mids", bufs=3))
            p1pool = ctx.enter_context(tc.tile_pool(name="p1", bufs=2, space="PSUM"))
            p2pool = ctx.enter_context(tc.tile_pool(name="p2", bufs=2, space="PSUM"))
            p5pool = ctx.enter_context(tc.tile_pool(name="p5", bufs=2, space="PSUM"))
            pSpool = ctx.enter_context(tc.tile_pool(name="pS", bufs=2, space="PSUM"))

            # fc1 consts first, then the head of the x stream (chunk 0 as
            # 4 k-slabs so the first matmuls begin earlier), then the
            # remaining consts, then the rest of the stream.
            c16_sb = singles.tile([128, C16_N], F16)
            nc.sync.dma_start(out=c16_sb, in_=c16[:, :])
            c32_sb = singles.tile([128, C32_N], F32)
            cb_sb = singles.tile([128, CB_N], BF16)

            xts = []
            for c in range(nch):
                xt = xpool.tile([128, 2, KS, CH], F16, tag="x", name="x_t")
                nslab = {0: 4}.get(c, 1)
                ks_per = KS // nslab
                for s in range(nslab):
                    sl = slice(s * ks_per, (s + 1) * ks_per)
                    nc.sync.dma_start(out=xt[:, :, sl, :],
                                      in_=xp[:, c, :, sl, :])
                xts.append(xt)
                if c == 0:
                    nc.sync.dma_start(out=c32_sb, in_=c32[:, :])
                if c == 4:
                    nc.sync.dma_start(out=cb_sb, in_=cb[:, :])

            cs1_s = c32_sb[0:H1, C32_CS1:C32_CS1 + 1]
            cs1_t = c32_sb[0:H1, C32_CS1 + 1:C32_CS1 + 2]
            cs2_s = c32_sb[32:32 + H2, C32_CS2:C32_CS2 + 1]
            cs2_t = c32_sb[32:32 + H2, C32_CS2 + 1:C32_CS2 + 2]
            b5_c = c32_sb[0:O, C32_B5:C32_B5 + 1]
            w2_sb = c16_sb[0:H1, C16_W2:C16_W2 + H2]
            w5_sb = c16_sb[32:32 + H2, C16_W5:C16_W5 + O]
            ones_sb = cb_sb[0:O, CB_ONES:CB_ONES + O]

            olog = singles.tile([O, bpc], F32)

            # PE warmup: dummy matmuls gated only on the consts DMA keep the
            # HAM activity window busy while the first x slabs land, so real
            # work starts at 2.4GHz. Output goes to the pS ring (never read).
            warm = pSpool.tile([O, CH], F32, tag="psS", name="warm")
            for _ in range(10):
                nc.tensor.matmul(warm[:, 0:384], lhsT=c16_sb[:, 0:O],
                                 rhs=c16_sb[:, 0:384], start=True, stop=True)

            # work items: (chunk, col_lo, col_hi); last two chunks split in
            # half to shorten the drain chain after the DMA stream ends.
            items = [(c, 0, CH) for c in range(nch - 2)]
            items.append((nch - 2, 0, CH // 2))
            items.append((nch - 2, CH // 2, CH))
            items.append((nch - 1, 0, CH // 2))
            items.append((nch - 1, CH // 2, 3 * CH // 4))
            items.append((nch - 1, 3 * CH // 4, CH))

            def fc1(it):
                c, lo, hi = it
                xt = xts[c]
                ps1 = p1pool.tile([H1, CH], F32, tag="ps1", name="ps1")[:, :hi - lo]
                last = None
                for k in range(KS):
                    w1k = c16_sb[:, C16_W1 + k * H1:C16_W1 + (k + 1) * H1]
                    nc.tensor.matmul(ps1, lhsT=w1k, rhs=xt[:, 0, k, lo:hi],
                                     start=(k == 0), stop=False)
                    last = nc.tensor.matmul(ps1, lhsT=w1k,
                                            rhs=xt[:, 1, k, lo:hi],
                                            start=False, stop=(k == KS - 1))
                return ps1, last

            def s_exp(ps5, it):
                """exp (bf16) + DVE copy of the logits to SBUF (frees ps5)."""
                n = it[2] - it[1]
                e = mids.tile([O, CH], BF16, tag="e", name="e")[:, :n]
                nc.scalar.activation(e, ps5, AF.Exp, bias=b5_c)
                lg = mids.tile([O, CH], F32, tag="lg", name="lg", bufs=4)[:, :n]
                nc.vector.tensor_copy(lg, ps5)
                return e, lg

            def s_sum(e, it):
                n = it[2] - it[1]
                psS = pSpool.tile([O, CH], F32, tag="psS", name="psS")[:, :n]
                h = nc.tensor.matmul(psS, lhsT=ones_sb, rhs=e, start=True,
                                     stop=True)
                return psS, h

            def s_fin(psS, lg, it):
                """ln -> (logits+b5)-lse -> store."""
                c, lo, hi = it
                n = hi - lo
                lse = mids.tile([O, CH], F32, tag="lse", name="lse")[:, :n]
                nc.scalar.activation(lse, psS, AF.Ln)
                oslice = olog[:, c * CH + lo:c * CH + hi]
                nc.vector.scalar_tensor_tensor(
                    out=oslice, in0=lg, scalar=b5_c, in1=lse,
                    op0=OP.add, op1=OP.subtract)
                if hi == CH:  # store once per chunk; SWDGE keeps the sync
                    # HWDGE ring free for the x stream, except the last two
                    # chunks where the SP queue is already drained
                    eng = nc.sync if c >= nch - 2 else nc.gpsimd
                    eng.dma_start(out=y[:, c * CH:(c + 1) * CH],
                                  in_=olog[:, c * CH:(c + 1) * CH])

            # software pipeline, deep lags so every cross-engine input is a
            # full iteration old (ACT's ~3us op block completes during the
            # producer's next fc1, so the PE never waits on ACT):
            #   PE:  fc1(i) fc2(i-2) fc5(i-4) sum(i-6)
            #   ACT: sign1(i-1) sign2(i-3) exp(i-5) ln(i-7)
            #   DVE: copy(i-5) stt(i-7); store(i-7) on gpsimd
            ps1s, y1s, ps2s, y2s, ps5s, es, lgs, pSs = ({} for _ in range(8))
            ni = len(items)

            def sign1(ps1, it):
                n = it[2] - it[1]
                y1 = mids.tile([H1, CH], F16, tag="y1", name="y1")[:, :n]
                nc.scalar.activation(y1, ps1, AF.Sign, bias=cs1_t, scale=cs1_s)
                return y1

            def fc2(y1, it):
                n = it[2] - it[1]
                ps2 = p2pool.tile([32 + H2, CH], F32, tag="ps2",
                                  name="ps2")[32:32 + H2, :n]
                h = nc.tensor.matmul(ps2, lhsT=w2_sb, rhs=y1, start=True,
                                     stop=True)
                return ps2, h

            def sign2(ps2, it):
                n = it[2] - it[1]
                y2 = mids.tile([32 + H2, CH], F16, tag="y2",
                               name="y2")[32:32 + H2, :n]
                nc.scalar.activation(y2, ps2, AF.Sign, bias=cs2_t, scale=cs2_s)
                return y2

            def fc5(y2, it):
                n = it[2] - it[1]
                ps5 = p5pool.tile([O, CH], F32, tag="ps5", name="ps5")[:, :n]
                h = nc.tensor.matmul(ps5, lhsT=w5_sb, rhs=y2, start=True,
                                     stop=True)
                return ps5, h

            fc1_last, fc2_h, fc5_h, sum_h = {}, {}, {}, {}
            for i in range(ni + 8):
                if i < ni:
                    ps1s[i], fc1_last[i] = fc1(items[i])
                if 0 <= i - 1 < ni:
                    y1s[i - 1] = sign1(ps1s.pop(i - 1), items[i - 1])
                if 0 <= i - 2 < ni:
                    ps2s[i - 2], fc2_h[i - 2] = fc2(y1s.pop(i - 2),
                                                    items[i - 2])
                if 0 <= i - 3 < ni:
                    y2s[i - 3] = sign2(ps2s.pop(i - 3), items[i - 3])
                if 0 <= i - 4 < ni:
                    ps5s[i - 4], fc5_h[i - 4] = fc5(y2s.pop(i - 4),
                                                    items[i - 4])
                if 0 <= i - 5 < ni:
                    es[i - 5], lgs[i - 5] = s_exp(ps5s.pop(i - 5), items[i - 5])
                if 0 <= i - 6 < ni:
                    pSs[i - 6], sum_h[i - 6] = s_sum(es.pop(i - 6),
                                                     items[i - 6])
                if 0 <= i - 7 < ni:
                    s_fin(pSs.pop(i - 7), lgs.pop(i - 7), items[i - 7])
                if i >= ni:
                    # dummy weight loads keep the PE HAM activity window
                    # busy through the flush ping-pong (no PSUM writes, no
                    # consumers -- correctness-neutral)
                    for _ in range(4):
                        nc.tensor.ldweights(weights=c16_sb[:, 0:H1])

            # Artificial PE ordering deps: the scheduler otherwise packs
            # fc2(c)/fc5(c)/sum(c) directly behind fc1(c), recreating a
            # zero-slack PE<->ACT serial chain (the PE then stalls ~1.5us per
            # chunk waiting on sign1/sign2).  Forcing them after later fc1
            # blocks gives every ACT stage a full fc1-block of slack.
            import bass_rust as _br
            _dep = _br.DependencyInfo(sync=True, no_sync=False)
            for c, h in fc2_h.items():
                if c + 1 in fc1_last:
                    h.ins.add_dependency(fc1_last[c + 1].ins.name, _dep)
            for c, h in fc5_h.items():
                if c + 2 in fc1_last:
                    h.ins.add_dependency(fc1_last[c + 2].ins.name, _dep)
            for c, h in sum_h.items():
                if c + 3 in fc1_last:
                    h.ins.add_dependency(fc1_last[c + 3].ins.name, _dep)

    nc.finalize()
    return nc


def _prep_inputs(x, w1, b1, g1, be1, m1, v1, w2, b2, g2, be2, m2, v2, w5, b5,
                 bpc: int = BPC, n_cores: int = N_CORES):
    f64 = np.float64
    w1s = np.where(w1 >= 0, 1.0, -1.0).astype(np.float32)
    w2s = np.where(w2 >= 0, 1.0, -1.0).astype(np.float32)
    w5s = np.where(w5 >= 0, 1.0, -1.0).astype(np.float32)

    w1t = np.ascontiguousarray(w1s.T)                              # [D, H1]
    w1sw = w1t.reshape(KS, 128, H1).transpose(1, 0, 2)             # [128,KS,H1]

    c16 = np.zeros((128, C16_N), dtype=np.float16)
    c16[:, C16_W1:C16_W1 + KS * H1] = w1sw.reshape(128, KS * H1)
    c16[0:H1, C16_W2:C16_W2 + H2] = w2s.T.astype(np.float16)
    c16[32:32 + H2, C16_W5:C16_W5 + O] = w5s.T.astype(np.float16)

    s1 = (g1.astype(f64) / np.sqrt(v1.astype(f64) + EPS))
    t1 = s1 * (b1.astype(f64) - m1.astype(f64)) + be1.astype(f64)
    s2 = (g2.astype(f64) / np.sqrt(v2.astype(f64) + EPS))
    t2 = s2 * (b2.astype(f64) - m2.astype(f64)) + be2.astype(f64)

    c32 = np.zeros((128, C32_N), dtype=np.float32)
    c32[0:H1, C32_CS1] = s1 / LO   # psum carries 2048*h1
    c32[0:H1, C32_CS1 + 1] = t1
    c32[32:32 + H2, C32_CS2] = s2
    c32[32:32 + H2, C32_CS2 + 1] = t2
    c32[0:O, C32_B5] = b5.astype(np.float32)

    import ml_dtypes
    cbb = np.zeros((128, CB_N), dtype=ml_dtypes.bfloat16)
    cbb[0:O, CB_ONES:CB_ONES + O] = 1.0

    x = np.asarray(x, dtype=np.float32)
    xh = x.astype(np.float16)
    xhs = (xh.astype(np.float32) * LO).astype(np.float16)  # exact scale
    xl = ((x - xh.astype(np.float32)) * LO).astype(np.float16)

    nch = bpc // CH

    def swizzle(a):  # [bpc, D] -> [128, nch, KS, CH]
        return np.ascontiguousarray(
            a.T.reshape(KS, 128, nch, CH).transpose(1, 2, 0, 3))

    in_maps = []
    for c in range(n_cores):
        rs = slice(c * bpc, (c + 1) * bpc)
        xpk = np.stack([swizzle(xhs[rs]), swizzle(xl[rs])], axis=2)
        in_maps.append({
            "xp": np.ascontiguousarray(xpk),  # [128, nch, 2, KS, CH]
            "c16": c16, "c32": c32, "cb": cbb,
        })
    return in_maps


def _decode_output(y_dev: np.ndarray, bpc: int) -> np.ndarray:
    return np.ascontiguousarray(y_dev.T)


_CACHED = {}


def kernel(**inputs) -> np.ndarray:
    from concourse.bass_utils import run_bass_kernel_spmd

    in_maps = _prep_inputs(**inputs)
    if "nc" not in _CACHED:
        _CACHED["nc"] = build_bass()
    nc = _CACHED["nc"]
    res = run_bass_kernel_spmd(nc, in_maps, list(range(N_CORES)))
    out = np.empty((B, O), dtype=np.float32)
    for c in range(N_CORES):
        out[c * BPC:(c + 1) * BPC] = _decode_output(res.results[c]["y"], BPC)
    return out
